# revision 3
# baseline (speedup 1.0000x reference)
"""Cross-modal multi-head attention on 8 Trainium2 NeuronCores.

Reference computation (biases are zero in setup_inputs):
    proj(x, W) = x @ W                      per-modality shared projection
    t2v = softmax(tq @ vk.T / sqrt(D)) ; out1 = (t2v @ vv) reshaped [B,N,H*D]
    v2t = softmax(vq @ tk.T / sqrt(D))      returned as [B,H,Nv,Nt]

Folding the projections:  S1 = Xtq (tW vW^T) Xvk^T,  S2 = Xvq (vW tW^T) Xtk^T,
out1 = softmax(S1) Xvv vW.  Only one 128x128 matrix M1T = vW tW^T is needed
on-device (it is the lhsT for both B-matmuls), plus vW for the output proj.

Sharding: 32 independent (b,h) pairs; core c handles b=c//2, h=4*(c%2)+j.

Per pair on-device (everything [v|t] tiled by 128):
    xT        : PE transposes of tq,vk,vq,tk into [d, n] layout (f32r)
    B1 = M1 Xvk^T, B2 = M1 Xvq^T                       (f32r matmuls)
    S1T[v,t]  = B1^T tqT ;  E1T = exp(S1T/sqrt(D))     (ACT, f32r out)
    ZT[d,t]  += Xvv^T E1T  (PSUM accum over v tiles)
    CS[*,t]  += ones^T E1T (column sums, broadcast to all 128 partitions)
    out1      = (ZT * 1/CS)^T-slices @ vW              (f32 matmuls)
    S2[v,t]   = B2^T tkT ;  E2 = exp(S2/sqrt(D)), rowsum via ACT accum
    out2[v,t] = E2 * (1/rowsum)                        (DVE tensor_scalar)
"""

import numpy as np

B, N, H, D = 4, 1024, 8, 128
NCORES = 8
PAIRS = 4  # (b,h) pairs per core
P = 128
NT = N // P  # 8 tiles of 128
SCALE = float(1.0 / np.sqrt(D))

_prog_cache = {}


def _build_program():
    import concourse.bacc as bacc
    import concourse.tile as tile
    from concourse import mybir
    from concourse.masks import make_identity

    F32 = mybir.dt.float32
    F32R = mybir.dt.float32r
    EXP = mybir.ActivationFunctionType.Exp

    nc = bacc.Bacc()

    tq = nc.dram_tensor("tq", [PAIRS, N, D], F32, kind="ExternalInput")
    vk = nc.dram_tensor("vk", [PAIRS, N, D], F32, kind="ExternalInput")
    vq = nc.dram_tensor("vq", [PAIRS, N, D], F32, kind="ExternalInput")
    tk = nc.dram_tensor("tk", [PAIRS, N, D], F32, kind="ExternalInput")
    vv = nc.dram_tensor("vv", [PAIRS, N, D], F32, kind="ExternalInput")
    m1t = nc.dram_tensor("m1t", [D, D], F32, kind="ExternalInput")  # vW @ tW.T
    vw = nc.dram_tensor("vw", [D, D], F32, kind="ExternalInput")
    o1 = nc.dram_tensor("o1", [PAIRS, N, D], F32, kind="ExternalOutput")
    o2 = nc.dram_tensor("o2", [PAIRS, N, N], F32, kind="ExternalOutput")

    with tile.TileContext(nc) as tc:
        with (
            tc.tile_pool(name="consts", bufs=1) as consts,
            tc.tile_pool(name="loads", bufs=2) as loads,
            tc.tile_pool(name="casts", bufs=2) as casts,
            tc.tile_pool(name="xt", bufs=2) as xtp,
            tc.tile_pool(name="bmat", bufs=2) as bmat,
            tc.tile_pool(name="estream", bufs=3) as estream,
            tc.tile_pool(name="small", bufs=3) as small,
            tc.tile_pool(name="ostream", bufs=3) as ostream,
            tc.tile_pool(name="ps_stream", bufs=2, space="PSUM") as ps_stream,
            tc.tile_pool(name="ps_acc", bufs=1, space="PSUM") as ps_acc,
        ):
            ident_f = consts.tile([P, P], F32)
            make_identity(nc, ident_f)
            ident = consts.tile([P, P], F32R)
            nc.gpsimd.tensor_copy(out=ident, in_=ident_f)

            ones_f = consts.tile([P, P], F32)
            nc.vector.memset(ones_f, 1.0)
            ones_r = consts.tile([P, P], F32R)
            nc.gpsimd.tensor_copy(out=ones_r, in_=ones_f)

            m1t_f = consts.tile([D, D], F32)
            nc.sync.dma_start(out=m1t_f, in_=m1t[:, :])
            m1t_r = consts.tile([D, D], F32R)
            nc.gpsimd.tensor_copy(out=m1t_r, in_=m1t_f)

            vw_sb = consts.tile([D, D], F32)
            nc.sync.dma_start(out=vw_sb, in_=vw[:, :])

            for j in range(PAIRS):
                # ---- loads: [N, D] as [128p, 8a, 128d] ----
                raw = {}
                for name, dram in (("tq", tq), ("vk", vk), ("vq", vq),
                                   ("tk", tk), ("vv", vv)):
                    t = loads.tile([P, NT, D], F32, tag=f"ld_{name}")
                    nc.sync.dma_start(
                        out=t, in_=dram[j].rearrange("(a p) d -> p a d", p=P))
                    raw[name] = t

                # ---- round to f32r (gpsimd; also joins DMA waits) ----
                rnd = {}
                for name in ("tq", "vk", "vq", "tk", "vv"):
                    t = casts.tile([P, NT, D], F32R, tag=f"r_{name}")
                    nc.gpsimd.tensor_copy(out=t, in_=raw[name])
                    rnd[name] = t

                # ---- PE transposes into [d, n] layout ----
                xT = {}
                for name in ("tq", "vk", "vq", "tk"):
                    dst = xtp.tile([P, NT, P], F32R, tag=f"xT_{name}")
                    for a in range(NT):
                        tp = ps_stream.tile([P, P], F32R, tag="s")
                        nc.tensor.transpose(tp, rnd[name][:, a, :], ident)
                        nc.vector.tensor_copy(out=dst[:, a, :], in_=tp)
                    xT[name] = dst

                # ---- B1 = M1 vkT, B2 = M1 vqT  (lhsT = M1^T = m1t) ----
                Bsb = {}
                for bn, src in (("B1", "vk"), ("B2", "vq")):
                    ps = ps_stream.tile([P, N], F32, tag="s")
                    for h2 in range(2):
                        nc.tensor.matmul(
                            ps[:, h2 * 512:(h2 + 1) * 512], m1t_r,
                            xT[src].rearrange("p a d -> p (a d)")[:, h2 * 512:(h2 + 1) * 512],
                            start=True, stop=True)
                    sb = bmat.tile([P, N], F32R, tag=bn)
                    nc.vector.tensor_copy(out=sb, in_=ps)
                    Bsb[bn] = sb

                tqT_flat = xT["tq"].rearrange("p a d -> p (a d)")
                tkT_flat = xT["tk"].rearrange("p a d -> p (a d)")

                # ---- phase L: t2v scores transposed, streamed over v tiles ----
                zt_ps = ps_acc.tile([P, N], F32, tag="zt")
                cs_ps = ps_acc.tile([P, N], F32, tag="cs")
                for vt in range(NT):
                    s1 = ps_stream.tile([P, N], F32, tag="s")
                    for h2 in range(2):
                        nc.tensor.matmul(
                            s1[:, h2 * 512:(h2 + 1) * 512],
                            Bsb["B1"][:, vt * P:(vt + 1) * P],
                            tqT_flat[:, h2 * 512:(h2 + 1) * 512],
                            start=True, stop=True)
                    e1 = estream.tile([P, N], F32R, tag="e1")
                    nc.scalar.activation(out=e1, in_=s1, func=EXP, scale=SCALE)
                    for h2 in range(2):
                        nc.tensor.matmul(
                            zt_ps[:, h2 * 512:(h2 + 1) * 512],
                            rnd["vv"][:, vt, :],
                            e1[:, h2 * 512:(h2 + 1) * 512],
                            start=(vt == 0), stop=(vt == NT - 1))
                        nc.tensor.matmul(
                            cs_ps[:, h2 * 512:(h2 + 1) * 512],
                            ones_r,
                            e1[:, h2 * 512:(h2 + 1) * 512],
                            start=(vt == 0), stop=(vt == NT - 1))

                # ---- phase L epilogue: normalize ZT, project with vW ----
                r1 = small.tile([P, N], F32, tag="r1")
                nc.vector.reciprocal(out=r1, in_=cs_ps)
                zts = small.tile([P, N], F32, tag="zts")
                nc.vector.tensor_mul(zts, zt_ps, r1)
                o1_sb = ostream.tile([P, NT, P], F32, tag="o1")
                for tt in range(NT):
                    op = ps_stream.tile([P, P], F32, tag="s")
                    nc.tensor.matmul(op, zts[:, tt * P:(tt + 1) * P], vw_sb,
                                     start=True, stop=True)
                    nc.scalar.copy(out=o1_sb[:, tt, :], in_=op)
                nc.sync.dma_start(
                    out=o1[j].rearrange("(a p) d -> p a d", p=P), in_=o1_sb)

                # ---- phase R: v2t attention (the big output) ----
                for vt in range(NT):
                    s2 = ps_stream.tile([P, N], F32, tag="s")
                    for h2 in range(2):
                        nc.tensor.matmul(
                            s2[:, h2 * 512:(h2 + 1) * 512],
                            Bsb["B2"][:, vt * P:(vt + 1) * P],
                            tkT_flat[:, h2 * 512:(h2 + 1) * 512],
                            start=True, stop=True)
                    e2 = estream.tile([P, N], F32, tag="e2")
                    s2sum = small.tile([P, 1], F32, tag="s2sum")
                    nc.scalar.activation(out=e2, in_=s2, func=EXP, scale=SCALE,
                                         accum_out=s2sum)
                    r2 = small.tile([P, 1], F32, tag="r2")
                    nc.vector.reciprocal(out=r2, in_=s2sum)
                    o2_sb = ostream.tile([P, N], F32, tag="o2")
                    nc.vector.tensor_scalar_mul(o2_sb, e2, r2)
                    nc.sync.dma_start(out=o2[j, vt * P:(vt + 1) * P, :], in_=o2_sb)

    nc.compile()
    return nc


def _get_program():
    if "nc" not in _prog_cache:
        _prog_cache["nc"] = _build_program()
    return _prog_cache["nc"]


def _get_executable():
    """Build (once) a persistent jitted shard_map executable over 8 cores.

    Mirrors bass2jax.run_bass_via_pjrt's multi-core path, but hoists the
    jax.jit out so repeated kernel() calls skip retracing/XLA recompile.
    """
    if "exe" in _prog_cache:
        return _prog_cache["exe"]

    import jax
    from jax.experimental.shard_map import shard_map
    from jax.sharding import Mesh, PartitionSpec
    from concourse import bass2jax, mybir

    nc = _get_program()
    bass2jax.install_neuronx_cc_hook()

    partition_name = nc.partition_id_tensor.name if nc.partition_id_tensor else None
    in_names, out_names, out_avals = [], [], []
    for alloc in nc.m.functions[0].allocations:
        if not isinstance(alloc, mybir.MemoryLocationSet):
            continue
        name = alloc.memorylocations[0].name
        if alloc.kind == "ExternalInput":
            if name != partition_name:
                in_names.append(name)
        elif alloc.kind == "ExternalOutput":
            out_names.append(name)
            out_avals.append(jax.core.ShapedArray(
                tuple(alloc.tensor_shape), mybir.dt.np(alloc.dtype)))
    n_params = len(in_names)
    n_outs = len(out_avals)
    all_in_names = in_names + out_names
    if partition_name is not None:
        all_in_names = all_in_names + [partition_name]

    def _body(*args):
        operands = list(args)
        if partition_name is not None:
            operands.append(bass2jax.partition_id_tensor())
        outs = bass2jax._bass_exec_p.bind(
            *operands,
            out_avals=tuple(out_avals),
            in_names=tuple(all_in_names),
            out_names=tuple(out_names),
            lowering_input_output_aliases=(),
            sim_require_finite=True,
            sim_require_nnan=True,
            nc=nc,
        )
        return tuple(outs)

    devices = jax.devices()[:NCORES]
    mesh = Mesh(np.asarray(devices), ("core",))
    in_specs = (PartitionSpec("core"),) * (n_params + n_outs)
    out_specs = (PartitionSpec("core"),) * n_outs
    donate = tuple(range(n_params, n_params + n_outs))
    sharded = jax.jit(
        shard_map(_body, mesh=mesh, in_specs=in_specs, out_specs=out_specs,
                  check_rep=False),
        donate_argnums=donate, keep_unused=True,
    )
    exe = (sharded, in_names, out_names,
           [tuple(a.shape) for a in out_avals], [a.dtype for a in out_avals])
    _prog_cache["exe"] = exe
    return exe


def _numpy_fallback(visual_query, visual_key, visual_value,
                    text_query, text_key, text_value, vW, vb, tW, tb):
    proj = lambda x, W, bb: np.einsum("bnhd,de->bnhe", x, W) + bb
    sw = lambda x: np.swapaxes(x, 1, 2)
    vq = sw(proj(visual_query, vW, vb))
    vk = sw(proj(visual_key, vW, vb))
    vv = sw(proj(visual_value, vW, vb))
    tq = sw(proj(text_query, tW, tb))
    tk = sw(proj(text_key, tW, tb))

    def softmax(x):
        m = x.max(-1, keepdims=True)
        e = np.exp(x - m)
        return e / e.sum(-1, keepdims=True)

    t2v = softmax(np.einsum("bhtd,bhvd->bhtv", tq, vk) * SCALE)
    v2t = softmax(np.einsum("bhvd,bhtd->bhvt", vq, tk) * SCALE)
    att = np.einsum("bhtv,bhvd->bhtd", t2v, vv)
    att = sw(att).reshape(B, N, H * D)
    return att.astype(np.float32), v2t.astype(np.float32)


def kernel(visual_query, visual_key, visual_value,
           text_query, text_key, text_value, vW, vb, tW, tb):
    visual_query = np.asarray(visual_query, dtype=np.float32)
    visual_key = np.asarray(visual_key, dtype=np.float32)
    visual_value = np.asarray(visual_value, dtype=np.float32)
    text_query = np.asarray(text_query, dtype=np.float32)
    text_key = np.asarray(text_key, dtype=np.float32)
    vW = np.asarray(vW, dtype=np.float32)
    tW = np.asarray(tW, dtype=np.float32)
    vb = np.asarray(vb, dtype=np.float32)
    tb = np.asarray(tb, dtype=np.float32)

    if np.any(vb) or np.any(tb):
        # the folded-projection kernel assumes zero biases (true for this
        # problem's setup_inputs); fall back to exact numpy otherwise
        return _numpy_fallback(visual_query, visual_key, visual_value,
                               text_query, text_key, text_value,
                               vW, vb, tW, tb)

    # Global arrays over pair index g = 8b + h, which equals 4*core + j for
    # core = 2b + h//4, j = h%4 — so shard_map's axis-0 split along 32 pairs
    # lands pairs on the intended cores with no per-core slicing.
    def by_pair(x):
        return np.ascontiguousarray(x.transpose(0, 2, 1, 3)).reshape(
            NCORES * PAIRS, N, D)

    g_in = {
        "tq": by_pair(text_query),
        "vk": by_pair(visual_key),
        "vq": by_pair(visual_query),
        "tk": by_pair(text_key),
        "vv": by_pair(visual_value),
        "m1t": np.ascontiguousarray(np.tile(vW @ tW.T, (NCORES, 1))),
        "vw": np.ascontiguousarray(np.tile(vW, (NCORES, 1))),
    }

    sharded, in_names, out_names, out_shapes, out_dtypes = _get_executable()
    zeros = [np.zeros((NCORES * s[0],) + s[1:], d)
             for s, d in zip(out_shapes, out_dtypes)]
    outs = sharded(*[g_in[n] for n in in_names], *zeros)
    res = {n: np.asarray(o) for n, o in zip(out_names, outs)}

    att = res["o1"].reshape(B, H, N, D)
    v2t = res["o2"].reshape(B, H, N, N)
    att_fs = np.ascontiguousarray(att.transpose(0, 2, 1, 3)).reshape(B, N, H * D)
    return att_fs, v2t
